# revision 1
# baseline (speedup 1.0000x reference)
"""Deformable-DETR encoder layer on 8 trn2 NeuronCores (axon/jax).

Split: all dense tensor compute (q/value/offset/attn projections, softmax,
output projection, residual+LayerNorm, FFN) runs on the 8 NeuronCores via
a gather-free shard_map graph, data-parallel over (batch=2 x 4 token
chunks). The data-dependent bilinear gather-and-blend (no dense FLOPs,
pure indexed reads) runs between the two device phases in vectorized
numpy on host.

Phase 1 (device): q = src+pos; value/off/attn projections; softmax.
Host:            bilinear sample + attention-weighted reduction.
Phase 2 (device): out-proj + residual LN + FFN + LN.
"""
import functools

import jax
import jax.numpy as jnp
import numpy as np
from jax.experimental.shard_map import shard_map
from jax.sharding import Mesh, PartitionSpec as P

SHAPES = ((100, 100), (50, 50), (25, 25), (13, 13))
B, D, NH, NL, NP, DFF = 2, 256, 8, 4, 4, 1024
DH = D // NH
S = sum(h * w for h, w in SHAPES)  # 13294
NCHUNK = 4
SPAD = ((S + NCHUNK - 1) // NCHUNK) * NCHUNK  # 13296
TC = SPAD // NCHUNK

_OFFSET_NORM = np.array([[w, h] for h, w in SHAPES], np.float32)
_LVL_START = np.cumsum([0] + [h * w for h, w in SHAPES])


def _layer_norm(x, w, b):
    m = x.mean(-1, keepdims=True)
    v = ((x - m) ** 2).mean(-1, keepdims=True)
    return (x - m) * jax.lax.rsqrt(v + 1e-5) * w + b


@functools.lru_cache(maxsize=1)
def _mesh():
    devs = np.array(jax.devices()[:8]).reshape(2, 4)
    return Mesh(devs, ("b", "c"))


@functools.lru_cache(maxsize=1)
def _phase1():
    mesh = _mesh()

    def body(src, pos, w_value, b_value, w_off, b_off, w_attn, b_attn):
        # block shapes [1,1,TC,D]
        s = src[0, 0]
        q = s + pos[0, 0]
        value = s @ w_value + b_value
        off = q @ w_off + b_off
        attn = jax.nn.softmax(
            (q @ w_attn + b_attn).reshape(TC, NH, NL * NP), axis=-1
        ).reshape(TC, NH * NL * NP)
        return (value.astype(jnp.bfloat16)[None, None],
                off.astype(jnp.bfloat16)[None, None],
                attn.astype(jnp.bfloat16)[None, None])

    fn = shard_map(
        body, mesh=mesh,
        in_specs=(P("b", "c"), P("b", "c")) + (P(None),) * 6,
        out_specs=(P("b", "c"), P("b", "c"), P("b", "c")),
        check_rep=False,
    )
    return jax.jit(fn)


@functools.lru_cache(maxsize=1)
def _phase2():
    mesh = _mesh()

    def body(acc, src, w_out, b_out, w_ff1, b_ff1, w_ff2, b_ff2,
             ln1_w, ln1_b, ln2_w, ln2_b):
        a = acc[0, 0].astype(jnp.float32)
        s = src[0, 0]
        ca = a @ w_out + b_out
        x = _layer_norm(s + ca, ln1_w, ln1_b)
        ff = jax.nn.relu(x @ w_ff1 + b_ff1) @ w_ff2 + b_ff2
        return _layer_norm(x + ff, ln2_w, ln2_b)[None, None]

    fn = shard_map(
        body, mesh=mesh,
        in_specs=(P("b", "c"), P("b", "c")) + (P(None),) * 10,
        out_specs=P("b", "c"),
        check_rep=False,
    )
    return jax.jit(fn)


@functools.lru_cache(maxsize=1)
def _sample_jit():
    cpu = jax.devices("cpu")[0]

    def fn(value, off, attn, ref):
        offv = off[:, :S].reshape(B, S, NH, NL, NP, 2)
        attnv = attn[:, :S].reshape(B, S, NH, NL, NP)
        out = jnp.zeros((B, S, NH, DH), jnp.float32)
        for l, (H_, W_) in enumerate(SHAPES):
            v = value[:, _LVL_START[l]:_LVL_START[l + 1]].reshape(
                B, H_ * W_, NH, DH)
            x = ref[:, :, l, 0, None, None] * W_ - 0.5 + offv[..., l, :, 0]
            y = ref[:, :, l, 1, None, None] * H_ - 0.5 + offv[..., l, :, 1]
            x0 = jnp.floor(x)
            y0 = jnp.floor(y)
            a = attnv[..., l, :]
            for dx, dy in ((0, 0), (1, 0), (0, 1), (1, 1)):
                xi = x0 + dx
                yi = y0 + dy
                w = (1.0 - jnp.abs(x - xi)) * (1.0 - jnp.abs(y - yi)) * a
                valid = (xi >= 0) & (xi < W_) & (yi >= 0) & (yi < H_)
                w = jnp.where(valid, w, 0.0)  # [B,S,NH,NP]
                idx = (jnp.clip(yi, 0, H_ - 1) * W_
                       + jnp.clip(xi, 0, W_ - 1)).astype(jnp.int32)
                idx_t = idx.transpose(0, 1, 3, 2).reshape(B, -1, NH, 1)
                g = jnp.take_along_axis(v, idx_t, axis=1).reshape(
                    B, S, NP, NH, DH)
                out = out + (g * w.transpose(0, 1, 3, 2)[..., None]).sum(2)
        return out.reshape(B, S, D)

    return jax.jit(fn, device=cpu)


def _sample_host(value, off, attn, ref):
    return np.asarray(_sample_jit()(value, off, attn, ref))


def kernel(**inputs):
    f32 = lambda k: np.asarray(inputs[k], np.float32)
    src, pos, ref = f32("src"), f32("pos"), f32("reference_points")

    pad = SPAD - S
    pad_tok = lambda a: np.concatenate(
        [a, np.zeros((B, pad) + a.shape[2:], a.dtype)], 1)
    from jax.sharding import NamedSharding
    mesh = _mesh()
    sh = NamedSharding(mesh, P("b", "c"))
    src_p = jax.device_put(pad_tok(src).reshape(B, NCHUNK, TC, D), sh)
    pos_p = jax.device_put(pad_tok(pos).reshape(B, NCHUNK, TC, D), sh)

    value, off, attn = _phase1()(
        src_p, pos_p, f32("w_value"), f32("b_value"),
        f32("w_off"), f32("b_off"), f32("w_attn"), f32("b_attn"))
    value = np.asarray(value).astype(np.float32).reshape(B, SPAD, D)[:, :S]
    off = np.asarray(off).astype(np.float32).reshape(B, SPAD, NH * NL * NP * 2)
    attn = np.asarray(attn).astype(np.float32).reshape(B, SPAD, NH * NL * NP)

    acc = _sample_host(value, off, attn, ref)

    acc_p = jax.device_put(
        pad_tok(acc).reshape(B, NCHUNK, TC, D).astype(jnp.bfloat16), sh)
    out = _phase2()(
        acc_p, src_p, f32("w_out"), f32("b_out"), f32("w_ff1"), f32("b_ff1"),
        f32("w_ff2"), f32("b_ff2"), f32("ln1_w"), f32("ln1_b"),
        f32("ln2_w"), f32("ln2_b"))
    return np.asarray(out).reshape(B, SPAD, D)[:, :S]



# revision 7
# speedup vs baseline: 5.7725x; 5.7725x over previous
"""Deformable-DETR encoder layer on 8 trn2 NeuronCores (axon/jax).

The axon tunnel runs at ~45 MB/s with ~80 ms per-dispatch RTT, so
wall-clock is dominated by wire bytes.  All compute runs on-device in
chained jitted shard_map calls (intermediates stay device-resident and
dispatches pipeline asynchronously):

  jit1: value/offset/attn projections, softmax, all_gather of the
        per-batch value table across the 4-chunk group, and per-level
        corner-fused flat gather indices + tent*attn weights.
  jit2 (x4 levels, one shared executable): the bilinear sample as a
        single take_along_axis row-gather per level — the only gather
        formulation the walrus backend compiles reliably; anything with
        multiple/fancier gathers per module crashes its indirect-DMA
        codegen.  Indices are pre-offset so all levels share one jaxpr.
  jit3: partial-acc sum, output projection + residual LayerNorm + FFN +
        LayerNorm.

Wire format: src/pos packed bf16 and sharded (batch=2 x 4 token chunks),
weights uploaded 1/8-sharded and all_gathered on-device, output bf16.
Tent weights at clamped patch positions reproduce grid_sample's
zero-padding semantics exactly.
"""
import functools

import numpy as np
import jax
import jax.numpy as jnp
import ml_dtypes
from jax.experimental.shard_map import shard_map
from jax.sharding import Mesh, NamedSharding, PartitionSpec as P

SHAPES = ((100, 100), (50, 50), (25, 25), (13, 13))
B, D, NH, NL, NP, DFF = 2, 256, 8, 4, 4, 1024
DH = D // NH
S = sum(h * w for h, w in SHAPES)  # 13294
NCHUNK = 4
SPAD = ((S + NCHUNK - 1) // NCHUNK) * NCHUNK  # 13296
T = SPAD // NCHUNK  # 3324
LVL_START = (0, 10000, 12500, 13125)
BF16 = ml_dtypes.bfloat16
NC4 = NP * 4  # points x corners per (token, head, level)

WSPEC = (
    ("w_value", D, D),
    ("w_off", D, NH * NL * NP * 2),
    ("w_attn", D, NH * NL * NP),
    ("w_out", D, D),
    ("w_ff1", D, DFF),
    ("w_ff2", DFF, D),
)
BSPEC = (
    ("b_value", D), ("b_off", NH * NL * NP * 2), ("b_attn", NH * NL * NP),
    ("b_out", D), ("b_ff1", DFF), ("b_ff2", D),
    ("ln1_w", D), ("ln1_b", D), ("ln2_w", D), ("ln2_b", D),
)


def _unpack_w(wg):
    ws, o = {}, 0
    for name, r, c in WSPEC:
        n = (r // 8) * c
        ws[name] = wg[:, o:o + n].reshape(r, c)
        o += n
    return ws


def _unpack_b(bias):
    bs, o = {}, 0
    for name, n in BSPEC:
        bs[name] = bias[o:o + n]
        o += n
    return bs


def _layer_norm(x, w, b):
    m = x.mean(-1, keepdims=True)
    v = ((x - m) ** 2).mean(-1, keepdims=True)
    return (x - m) * jax.lax.rsqrt(v + 1e-5) * w + b


@functools.lru_cache(maxsize=1)
def _mesh():
    devs = np.array(jax.devices()[:8]).reshape(2, 4)
    return Mesh(devs, ("b", "c"))


@functools.lru_cache(maxsize=1)
def _fn1():
    mesh = _mesh()

    def body(tp, refp, wloc, bias):
        wg = jax.lax.all_gather(wloc, ("b", "c"), axis=0, tiled=True)
        ws = _unpack_w(wg)
        bs = _unpack_b(bias)
        f32 = jnp.float32

        src = tp[0, 0, :, :D]
        pos = tp[0, 0, :, D:]
        ref = refp[0, 0].reshape(T, NL, 2)

        value_c = (
            jnp.dot(src, ws["w_value"], preferred_element_type=f32)
            + bs["b_value"]
        ).astype(jnp.bfloat16)
        value = jax.lax.all_gather(value_c, "c", axis=0, tiled=True)
        value = value.reshape(SPAD, NH, DH)

        q = src + pos
        off = (
            jnp.dot(q, ws["w_off"], preferred_element_type=f32) + bs["b_off"]
        ).reshape(T, NH, NL, NP, 2)
        logits = (
            jnp.dot(q, ws["w_attn"], preferred_element_type=f32)
            + bs["b_attn"]
        ).reshape(T, NH, NL * NP)
        e = jnp.exp(logits - logits.max(-1, keepdims=True))
        attn = (e / e.sum(-1, keepdims=True)).reshape(T, NH, NL, NP)

        # Per level: clamped 2x2 patch positions; tent weights at the
        # clamped positions reproduce zero-padding bilinear exactly.
        idxs, wgts = [], []
        di = jnp.arange(2, dtype=f32)
        for l, (H_, W_) in enumerate(SHAPES):
            x = ref[:, None, l, None, 0] * W_ - 0.5 + off[:, :, l, :, 0]
            y = ref[:, None, l, None, 1] * H_ - 0.5 + off[:, :, l, :, 1]
            p0x = jnp.clip(jnp.floor(x), 0, W_ - 2)  # [T, NH, NP]
            p0y = jnp.clip(jnp.floor(y), 0, H_ - 2)
            wx = jnp.maximum(
                0.0, 1.0 - jnp.abs(x[..., None] - p0x[..., None] - di)
            )  # [T, NH, NP, 2]
            wy = jnp.maximum(
                0.0, 1.0 - jnp.abs(y[..., None] - p0y[..., None] - di)
            )
            wgt = (
                wy[..., :, None] * wx[..., None, :]
                * attn[:, :, l, :, None, None]
            )  # [T, NH, NP, 2, 2]
            idx = (
                (p0y[..., None, None] + di[:, None]) * W_
                + p0x[..., None, None] + di[None, :]
            ) + float(LVL_START[l])  # [T, NH, NP, 2, 2]
            # -> [T, NP, 2, 2, NH] -> rows-major [T*NC4, NH]
            idxs.append(
                idx.astype(jnp.int32).transpose(0, 2, 3, 4, 1)
                .reshape(T * NC4, NH)[None, None]
            )
            wgts.append(
                wgt.transpose(0, 2, 3, 4, 1).reshape(T, NC4, NH)[None, None]
            )
        return (value[None, None],) + tuple(idxs) + tuple(wgts)

    fn = shard_map(
        body, mesh=mesh,
        in_specs=(P("b", "c"), P("b", "c"), P(("b", "c")), P()),
        out_specs=(P("b", "c"),) * 9,
        check_rep=False,
    )
    return jax.jit(fn)


@functools.lru_cache(maxsize=1)
def _fn2():
    mesh = _mesh()

    def body(value, idxc, wgtc):
        g = jnp.take_along_axis(
            value[0, 0], idxc[0, 0][:, :, None], axis=0
        )  # [T*NC4, NH, DH] bf16
        acc = (
            g.reshape(T, NC4, NH, DH).astype(jnp.float32)
            * wgtc[0, 0][..., None]
        ).sum(1)  # [T, NH, DH] f32
        return acc.reshape(T, D)[None, None]

    fn = shard_map(
        body, mesh=mesh,
        in_specs=(P("b", "c"),) * 3,
        out_specs=P("b", "c"),
        check_rep=False,
    )
    return jax.jit(fn)


@functools.lru_cache(maxsize=1)
def _fn3():
    mesh = _mesh()

    def body(a0, a1, a2, a3, tp, wloc, bias):
        wg = jax.lax.all_gather(wloc, ("b", "c"), axis=0, tiled=True)
        ws = _unpack_w(wg)
        bs = _unpack_b(bias)
        f32 = jnp.float32
        src = tp[0, 0, :, :D]
        acc = (a0[0, 0] + a1[0, 0]) + (a2[0, 0] + a3[0, 0])
        ca = (
            jnp.dot(
                acc.astype(jnp.bfloat16), ws["w_out"],
                preferred_element_type=f32,
            )
            + bs["b_out"]
        )
        x1 = _layer_norm(src.astype(f32) + ca, bs["ln1_w"], bs["ln1_b"])
        h = (
            jnp.dot(
                x1.astype(jnp.bfloat16), ws["w_ff1"],
                preferred_element_type=f32,
            )
            + bs["b_ff1"]
        )
        h = jnp.maximum(h, 0.0).astype(jnp.bfloat16)
        ff = jnp.dot(h, ws["w_ff2"], preferred_element_type=f32) + bs["b_ff2"]
        out = _layer_norm(x1 + ff, bs["ln2_w"], bs["ln2_b"])
        return out.astype(jnp.bfloat16)[None, None]

    fn = shard_map(
        body, mesh=mesh,
        in_specs=(P("b", "c"),) * 5 + (P(("b", "c")), P()),
        out_specs=P("b", "c"),
        check_rep=False,
    )
    return jax.jit(fn)


def kernel(**inputs):
    f32 = lambda k: np.asarray(inputs[k], np.float32)
    src, pos = f32("src"), f32("pos")
    ref = f32("reference_points")

    tp = np.zeros((B, SPAD, 2 * D), BF16)
    tp[:, :S, :D] = src.astype(BF16)
    tp[:, :S, D:] = pos.astype(BF16)
    tp = tp.reshape(B, NCHUNK, T, 2 * D)
    refp = np.zeros((B, SPAD, NL * 2), np.float32)
    refp[:, :S] = ref.reshape(B, S, NL * 2)
    refp = refp.reshape(B, NCHUNK, T, NL * 2)
    wloc = np.concatenate(
        [f32(n).astype(BF16).reshape(8, (r // 8) * c) for n, r, c in WSPEC],
        axis=1,
    )
    bias = np.concatenate([f32(n) for n, _ in BSPEC])

    mesh = _mesh()
    sh_bc = NamedSharding(mesh, P("b", "c"))
    sh_w = NamedSharding(mesh, P(("b", "c")))
    sh_r = NamedSharding(mesh, P())
    tp_d = jax.device_put(tp, sh_bc)
    refp_d = jax.device_put(refp, sh_bc)
    wloc_d = jax.device_put(wloc, sh_w)
    bias_d = jax.device_put(bias, sh_r)

    o1 = _fn1()(tp_d, refp_d, wloc_d, bias_d)
    value, idxs, wgts = o1[0], o1[1:5], o1[5:9]
    f2 = _fn2()
    accs = [f2(value, idxs[l], wgts[l]) for l in range(NL)]
    out = _fn3()(*accs, tp_d, wloc_d, bias_d)
    res = np.asarray(out)  # [B, NCHUNK, T, D] bf16
    return res.reshape(B, SPAD, D)[:, :S].astype(np.float32)
